# revision 24
# baseline (speedup 1.0000x reference)
"""Longformer-style sliding-chunk self-attention for Trainium2 (Bass/Tile).

Problem: B=2, T=4096, E=768, H=12 heads (head dim 64), window chunk W=256.
  q = (x @ wq.T)/8, k = x @ wk.T, v = x @ wv.T  (per head)
  scores: each chunk of 256 queries attends to [prev, cur, next] chunks
  (3*256 = 768 keys, zero-padded at sequence ends, with triangular masks on
  the pad blocks), softmax over the 768 window, then probs @ V.

Sharding: 8 cores = 2 batches x 4 head-groups of 3 heads. Each core gets
x[b].T (pre-transposed on host), per-head weight slices (transposed on
host, with the 1/8 query scale folded into wq), and produces
out[b, :, g*192:(g+1)*192].

Per-core kernel (all layouts chosen so no on-chip transposes are needed):
  - Q.T, K.T computed in [head_dim, T] layout (PSUM out of matmuls with
    weight slices as the stationary operand, x.T streaming).
  - V computed in natural [T, head_dim] layout (x.T tiles stationary,
    wv.T streaming), stored with a ones-column appended (V_aug) so the
    P@V matmul also produces the softmax denominator for free.
  - scores are computed TRANSPOSED: S.T[key, q] = K.T_tile.T @ Q.T_chunk,
    softmax uses exp WITHOUT max subtraction (scores ~ N(0,1), max < ~7,
    exp is safe in fp32) so no partition-dim reduction is ever needed.
  - P@V runs transposed: ctx.T[s|denom, q] = V_aug.T @ expS.T with V_aug
    [k,65] stationary (cheap weight load) and expS.T [k,256] moving
    (full-rate float32r streaming); row 64 is the softmax denominator.
    Two PE transposes (65x128 -> 128x65) restore [q, s] layout, then
    reciprocal + tensor_scalar multiply normalize, and the result DMAs out.
  - boundary chunks: the zero-padded prev/next blocks have score 0, so
    exp(0)*mask = mask; the 0/1 mask tiles (precomputed on host) are used
    directly as the "expS" moving operand with a zeros+ones V_aug pad
    tile, which also fixes the denominator. No masking work in the kernel.
  - matmuls use float32r (fp32 via bf16 hi/lo replication in the PE):
    1 cycle/row when the moving dim >= 256 vs 4 cycles/row for plain fp32.
    Rel err vs the fp32 reference is ~4.6e-4 (plain fp32 was ~4e-6).
  - the attention loop is software-pipelined 3 deep (QK(c) | PV(c-1) |
    transpose+normalize(c-2)) because the PE executes in order and would
    otherwise stall on the ACT exp between QK(c) and PV(c).
"""

import math

import numpy as np

B, T, E, H, WIN = 2, 4096, 768, 12, 256
S = 64            # head dim
NH = 3            # heads per core
ET = E // 128     # 6 e-tiles
TT = T // 128     # 32 t-tiles
C = T // WIN      # 16 chunks
NCORES = 8
NCH = 8           # 512-wide column chunks for the projections
PROJN = T // NCH  # 512


def _build_module(loop_n=None, parts=("load", "vproj", "qkproj", "attn")):
    """Build + compile the per-core Bass module. Same program on all cores.

    parts: ablation control for timing experiments (kernel() always uses all).
    """
    from contextlib import ExitStack

    import concourse.mybir as mybir
    from concourse import bacc
    from concourse.tile import TileContext

    fp32 = mybir.dt.float32
    fp32r = mybir.dt.float32r
    Exp = mybir.ActivationFunctionType.Exp

    nc = bacc.Bacc("TRN2", target_bir_lowering=False, debug=False,
                   num_devices=NCORES)
    xT = nc.dram_tensor("xT", [E, T], fp32, kind="ExternalInput")
    wqk = nc.dram_tensor("wqk", [E, NH, 128], fp32, kind="ExternalInput")
    # wv is padded to 4*S=256 columns (last 64 zero) so the V projection's
    # moving dim is 256, which lets float32r run at 1 cycle/row.
    wv = nc.dram_tensor("wv", [E, 4 * S], fp32, kind="ExternalInput")
    masks = nc.dram_tensor("masks", [128, 4, WIN], fp32, kind="ExternalInput")
    ident = nc.dram_tensor("ident", [S + 1, S + 1], fp32, kind="ExternalInput")
    out = nc.dram_tensor("out", [T, NH * S], fp32, kind="ExternalOutput")

    def emit(tc, ctx):
        singles = ctx.enter_context(tc.tile_pool(name="singles", bufs=1))
        qk_pool = ctx.enter_context(tc.tile_pool(name="qk", bufs=1))
        st_pool = ctx.enter_context(tc.tile_pool(name="st", bufs=2, space="PSUM"))
        pv_pool = ctx.enter_context(tc.tile_pool(name="pv", bufs=2, space="PSUM"))
        ex_pool = ctx.enter_context(tc.tile_pool(name="ex", bufs=2))
        o_pool = ctx.enter_context(tc.tile_pool(name="o", bufs=4))
        cx_pool = ctx.enter_context(tc.tile_pool(name="cx", bufs=3))
        sm_pool = ctx.enter_context(tc.tile_pool(name="sm", bufs=4))

        # ---- persistent SBUF tensors ----
        xt = singles.tile([128, ET, T], fp32)            # x[b].T   96KB/part
        wqk_sb = singles.tile([128, ET, NH, 128], fp32)  # 9KB/part
        wv_sb = singles.tile([128, ET, 4 * S], fp32)     # 6KB/part
        mask_sb = singles.tile([128, 4, WIN], fp32)      # 4KB/part
        v3 = singles.tile([128, TT, NH, S + 1], fp32)    # V_aug  24.4KB/part
        vpad = singles.tile([128, S + 1], fp32)
        ident_sb = singles.tile([S + 1, S + 1], fp32)

        # ---- input loads ----
        # xt/wqk/wv feed float32r matmuls: the BIR verifier requires their
        # producers to emit float32r, so the loads are bitcast on both sides
        # (same 4-byte values; the PE does the hi/lo bf16 split at load).
        xT_r = xT.ap().bitcast(fp32r).rearrange("(a p) t -> a p t", p=128)
        if "load" in parts:
            for tq in range(4):
                for et in range(ET):
                    nc.sync.dma_start(out=xt[:, et, tq * 1024:(tq + 1) * 1024].bitcast(fp32r),
                                      in_=xT_r[et][:, tq * 1024:(tq + 1) * 1024])
        nc.sync.dma_start(out=wqk_sb[:].bitcast(fp32r),
                          in_=wqk.ap().bitcast(fp32r).rearrange("(a p) g m -> p a g m", p=128))
        nc.sync.dma_start(out=wv_sb[:].bitcast(fp32r),
                          in_=wv.ap().bitcast(fp32r).rearrange("(a p) m -> p a m", p=128))
        nc.sync.dma_start(out=mask_sb[:].bitcast(fp32r),
                          in_=masks.ap().bitcast(fp32r))
        nc.sync.dma_start(out=ident_sb[:], in_=ident.ap())
        # DVE memset cannot emit float32r; stage the constants in fp32 and
        # round through ACT copies (valid float32r producers).
        cst = singles.tile([128, S + 1 + TT * NH], fp32)
        nc.vector.memset(cst[:], 0.0)
        nc.vector.memset(cst[:, S:S + 1], 1.0)
        nc.vector.memset(cst[:, S + 1:], 1.0)
        nc.scalar.copy(out=vpad[:].bitcast(fp32r), in_=cst[:, 0:S + 1])
        nc.scalar.copy(
            out=v3[:, :, :, S:S + 1].bitcast(fp32r),
            in_=cst[:, S + 1:].rearrange("p (a g one) -> p a g one",
                                         a=TT, g=NH, one=1))

        # ---- V projection, all heads: V[t, s] (+ ones col) ----
        for tt in range(TT if "vproj" in parts else 0):
            pvv = pv_pool.tile([128, 4 * S], fp32, tag="ctx")
            for et in range(ET):
                nc.tensor.matmul(pvv[:],
                                 xt[:, et, tt * 128:(tt + 1) * 128].bitcast(fp32r),
                                 wv_sb[:, et, :].bitcast(fp32r),
                                 start=(et == 0), stop=(et == ET - 1))
            nc.vector.tensor_copy(
                out=v3[:, tt, :, 0:S].bitcast(fp32r),
                in_=pvv[:, 0:NH * S].rearrange("p (g s) -> p g s", g=NH))

        # ---- Q.T/K.T projections + attention ----
        # Heads 0 and 1 are projected TOGETHER: stationary [wq_h0|wq_h1]
        # (and [wk_h0|wk_h1]) is a full M=128 operand, so head 0 lands in
        # PSUM/SBUF partitions 0:64 and head 1 in 64:128. Attention matmuls
        # for head 1 then run with both operands on partitions 64:128
        # (tile_position auto-derives to row group 64). Head 2 is processed
        # FIRST with M=64 projections so it can reuse the same pair buffers.
        def proj_head2(qt, kt):
            for nch in range(NCH):
                sl = slice(nch * PROJN, (nch + 1) * PROJN)
                psq = pv_pool.tile([64, PROJN], fp32, tag="ctx")
                for et in range(ET):
                    nc.tensor.matmul(psq[:], wqk_sb[:, et, 2, 0:64].bitcast(fp32r),
                                     xt[:, et, sl].bitcast(fp32r),
                                     start=(et == 0), stop=(et == ET - 1))
                nc.scalar.copy(out=qt[:, sl].bitcast(fp32r), in_=psq[:])
                psk = pv_pool.tile([64, PROJN], fp32, tag="ctx")
                for et in range(ET):
                    nc.tensor.matmul(psk[:], wqk_sb[:, et, 2, 64:128].bitcast(fp32r),
                                     xt[:, et, sl].bitcast(fp32r),
                                     start=(et == 0), stop=(et == ET - 1))
                nc.vector.tensor_copy(out=kt[:, sl].bitcast(fp32r), in_=psk[:])

        def proj_pair(qt, kt):
            for nch in range(NCH):
                sl = slice(nch * PROJN, (nch + 1) * PROJN)
                psq = pv_pool.tile([128, PROJN], fp32, tag="ctx")
                for et in range(ET):
                    nc.tensor.matmul(psq[:], wqk_sb[:, et, 0, :].bitcast(fp32r),
                                     xt[:, et, sl].bitcast(fp32r),
                                     start=(et == 0), stop=(et == ET - 1))
                nc.scalar.copy(out=qt[:, sl].bitcast(fp32r), in_=psq[:])
                psk = pv_pool.tile([128, PROJN], fp32, tag="ctx")
                for et in range(ET):
                    nc.tensor.matmul(psk[:], wqk_sb[:, et, 1, :].bitcast(fp32r),
                                     xt[:, et, sl].bitcast(fp32r),
                                     start=(et == 0), stop=(et == ET - 1))
                nc.vector.tensor_copy(out=kt[:, sl].bitcast(fp32r), in_=psk[:])

        def attn_head(qt, kt, g):
            # Software-pipelined attention: the PE is in-order, so PV(c) right
            # after QK(c) would stall on exp(c). Emit QK(c+1) between exp(c)
            # and PV(c) so the PE always has independent matmuls in hand.
            def emit_qk(c):
                lo = 2 if c == 0 else 0        # first valid window key-tile
                hi = 4 if c == C - 1 else 6    # one past last valid
                stp = st_pool.tile([128, 6, WIN], fp32, tag="st")
                for w_i in range(lo, hi):
                    gk = (c - 1) * 2 + w_i
                    nc.tensor.matmul(stp[:, w_i, :],
                                     kt[:, gk * 128:(gk + 1) * 128].bitcast(fp32r),
                                     qt[:, c * WIN:(c + 1) * WIN].bitcast(fp32r),
                                     start=True, stop=True)
                ex = ex_pool.tile([128, 6, WIN], fp32)
                nc.scalar.activation(out=ex[:, lo:hi, :].bitcast(fp32r),
                                     in_=stp[:, lo:hi, :], func=Exp)
                return (c, lo, hi, ex)

            def emit_pv(state):
                # ctx.T[s|denom, q] = sum_k V_aug[k, s] * expS.T[k, q]:
                # V_aug stationary (65 cols -> cheap weight load), expS.T
                # moving (256 -> full-rate fp32r streaming).
                c, lo, hi, ex = state
                ctxT = pv_pool.tile([S + 1, WIN], fp32, tag="ctx")
                for w_i in range(6):
                    if w_i < lo:
                        sta, mov = vpad[:], mask_sb[:, w_i, :]
                    elif w_i >= hi:
                        sta, mov = vpad[:], mask_sb[:, 2 + (w_i - 4), :]
                    else:
                        gk = (c - 1) * 2 + w_i
                        sta, mov = v3[:, gk, g, :], ex[:, w_i, :]
                    nc.tensor.matmul(ctxT[:], sta.bitcast(fp32r),
                                     mov.bitcast(fp32r),
                                     start=(w_i == 0), stop=(w_i == 5))
                ctxs = cx_pool.tile([S + 1, WIN], fp32)
                nc.vector.tensor_copy(out=ctxs[:], in_=ctxT[:])
                return (c, ctxs)

            def emit_fin(state):
                c, ctxs = state
                tout = pv_pool.tile([128, 2, S + 1], fp32, tag="ctx")
                for qh in range(2):
                    nc.tensor.transpose(tout[:, qh, :],
                                        ctxs[:, qh * 128:(qh + 1) * 128],
                                        ident_sb[:])
                rc = sm_pool.tile([128, 2, 1], fp32)
                nc.vector.reciprocal(rc[:], tout[:, :, S:S + 1])
                ob = o_pool.tile([128, 2, S], fp32)
                for qh in range(2):
                    nc.vector.tensor_scalar_mul(ob[:, qh, :], tout[:, qh, 0:S],
                                                rc[:, qh, 0:1])
                nc.sync.dma_start(
                    out=out.ap()[c * WIN:(c + 1) * WIN, g * S:(g + 1) * S]
                        .rearrange("(q2 p) s -> p q2 s", p=128),
                    in_=ob[:])

            stages = []
            for c in range(C):
                stages.append(emit_qk(c))
                if len(stages) >= 2:
                    stages[-2] = emit_pv(stages[-2])
                if len(stages) >= 3:
                    emit_fin(stages.pop(0))
            stages[-1] = emit_pv(stages[-1])
            for s2 in stages:
                emit_fin(s2)

        # head 2 first (separate M=64 projections), then the 0/1 pair.
        qtA = qk_pool.tile([128, T], fp32, tag="qt")
        ktA = qk_pool.tile([128, T], fp32, tag="kt")
        if "qkproj" in parts:
            proj_head2(qtA[0:64, :], ktA[0:64, :])
        elif "attn" in parts:
            nc.vector.memset(qtA[:], 0.01)
            nc.vector.memset(ktA[:], 0.01)
        if "attn" in parts:
            attn_head(qtA[0:64, :], ktA[0:64, :], 2)
        qtB = qk_pool.tile([128, T], fp32, tag="qt")
        ktB = qk_pool.tile([128, T], fp32, tag="kt")
        if "qkproj" in parts:
            proj_pair(qtB, ktB)
        elif "attn" in parts:
            nc.vector.memset(qtB[:], 0.01)
            nc.vector.memset(ktB[:], 0.01)
        if "attn" in parts:
            attn_head(qtB[0:64, :], ktB[0:64, :], 0)
            attn_head(qtB[64:128, :], ktB[64:128, :], 1)

    with TileContext(nc) as tc:
        with ExitStack() as ctx:
            if loop_n is None:
                emit(tc, ctx)
            else:
                with tc.For_i(0, loop_n, 1):
                    emit(tc, ctx)
    nc.compile()
    return nc


def _make_masks():
    """0/1 multiplicative masks for the zero-padded prev/next blocks, in
    expS.T layout [key_within_tile, q]. Slots 0,1: chunk-0 prev tiles;
    slots 2,3: chunk-15 next tiles."""
    m = np.ones((128, 4, WIN), dtype=np.float32)
    p = np.arange(128)[:, None]
    q = np.arange(WIN)[None, :]
    for kt in range(2):
        k = kt * 128 + p
        m[:, kt, :] = np.where(q < WIN - k, 0.0, 1.0)
    for et in range(2):
        kn = et * 128 + p
        m[:, 2 + et, :] = np.where(q >= (WIN - 1) - kn, 0.0, 1.0)
    return m


def _prep_inputs(x, wq, wk, wv):
    """Host-side shard prep: per-core input dicts."""
    masks = _make_masks()
    xTb = [np.ascontiguousarray(x[b].T) for b in range(B)]
    wqs = wq.astype(np.float32) * np.float32(1.0 / math.sqrt(S))
    in_maps = []
    for core in range(NCORES):
        b, grp = divmod(core, 4)
        h0 = grp * NH
        # wqk slot 0 = [wq_h0 | wq_h1] (scaled), slot 1 = [wk_h0 | wk_h1],
        # slot 2 = [wq_h2 (scaled) | wk_h2] -- see proj_pair/proj_head2.
        wqk_np = np.empty((E, NH, 128), dtype=np.float32)
        wv_np = np.zeros((E, 4 * S), dtype=np.float32)
        for g in range(NH):
            h = h0 + g
            wv_np[:, g * S:(g + 1) * S] = wv[h * S:(h + 1) * S, :].T
        wqk_np[:, 0, 0:64] = wqs[h0 * S:(h0 + 1) * S, :].T
        wqk_np[:, 0, 64:128] = wqs[(h0 + 1) * S:(h0 + 2) * S, :].T
        wqk_np[:, 1, 0:64] = wk[h0 * S:(h0 + 1) * S, :].T
        wqk_np[:, 1, 64:128] = wk[(h0 + 1) * S:(h0 + 2) * S, :].T
        wqk_np[:, 2, 0:64] = wqs[(h0 + 2) * S:(h0 + 3) * S, :].T
        wqk_np[:, 2, 64:128] = wk[(h0 + 2) * S:(h0 + 3) * S, :].T
        in_maps.append({"xT": xTb[b], "wqk": wqk_np, "wv": wv_np,
                        "masks": masks,
                        "ident": np.eye(S + 1, dtype=np.float32)})
    return in_maps


class _Runner:
    """Compile once; execute many times via PJRT across the 8 cores."""

    def __init__(self, loop_n=None):
        import jax
        import concourse.mybir as mybir
        from concourse import bass2jax
        from jax.sharding import Mesh, PartitionSpec
        from jax.experimental.shard_map import shard_map

        self.jax = jax
        nc = _build_module(loop_n=loop_n)
        self.nc = nc
        bass2jax.install_neuronx_cc_hook()

        partition_name = (nc.partition_id_tensor.name
                          if nc.partition_id_tensor else None)
        in_names, out_names, out_avals = [], [], []
        for alloc in nc.m.functions[0].allocations:
            if not isinstance(alloc, mybir.MemoryLocationSet):
                continue
            name = alloc.memorylocations[0].name
            if alloc.kind == "ExternalInput":
                if name != partition_name:
                    in_names.append(name)
            elif alloc.kind == "ExternalOutput":
                out_names.append(name)
                out_avals.append(jax.core.ShapedArray(
                    tuple(alloc.tensor_shape), mybir.dt.np(alloc.dtype)))
        self.in_names = in_names
        self.out_names = out_names
        n_params = len(in_names)
        n_outs = len(out_names)
        self.out_avals = out_avals
        in_names_all = list(in_names) + list(out_names)
        if partition_name:
            in_names_all.append(partition_name)

        def _body(*args):
            operands = list(args)
            if partition_name is not None:
                operands.append(bass2jax.partition_id_tensor())
            outs = bass2jax._bass_exec_p.bind(
                *operands, out_avals=tuple(out_avals),
                in_names=tuple(in_names_all), out_names=tuple(out_names),
                lowering_input_output_aliases=(),
                sim_require_finite=True, sim_require_nnan=True, nc=nc)
            return tuple(outs)

        devices = jax.devices()[:NCORES]
        mesh = Mesh(np.asarray(devices), ("core",))
        self._fn = jax.jit(
            shard_map(_body, mesh=mesh,
                      in_specs=(PartitionSpec("core"),) * (n_params + n_outs),
                      out_specs=(PartitionSpec("core"),) * n_outs,
                      check_rep=False),
            keep_unused=True)

    def put_args(self, in_maps):
        concat_in = [np.concatenate([m[nm] for m in in_maps], axis=0)
                     for nm in self.in_names]
        concat_zero = [np.zeros((NCORES * a.shape[0], *a.shape[1:]), a.dtype)
                       for a in self.out_avals]
        return [self.jax.device_put(a) for a in concat_in + concat_zero]

    def run(self, args):
        res = self.jax.block_until_ready(self._fn(*args))
        return [np.asarray(r) for r in res]


_RUNNER = None


def kernel(x, wq, wk, wv):
    global _RUNNER
    x = np.asarray(x, dtype=np.float32)
    wq = np.asarray(wq, dtype=np.float32)
    wk = np.asarray(wk, dtype=np.float32)
    wv = np.asarray(wv, dtype=np.float32)
    if _RUNNER is None:
        _RUNNER = _Runner()
    in_maps = _prep_inputs(x, wq, wk, wv)
    args = _RUNNER.put_args(in_maps)
    outs = _RUNNER.run(args)
    o = outs[0].reshape(NCORES, T, NH * S)
    full = np.empty((B, T, E), dtype=np.float32)
    for core in range(NCORES):
        b, grp = divmod(core, 4)
        full[b, :, grp * NH * S:(grp + 1) * NH * S] = o[core]
    return full
